# revision 1
# baseline (speedup 1.0000x reference)
"""CMSA (cross-modal self-attention) model on 8 Trainium2 NeuronCores.

Model (B=4, C=256, H=W=64, N=4096, A=256):
  spatial = fixed 8-channel coordinate features            [B, 8, H, W]
  mm   = concat(images, flows, spatial)                    [B, 520, H, W]
  img_feat  = CMSA(mm,   img_w*)                           [B, 256, H, W]
  lang_feat = CMSA(flows, lang_w*)                         [B, 256, H, W]
  out = conv1x1(concat(img_feat, lang_feat, spatial), fus) [B, 256, H, W]
where CMSA(x) = wo @ softmax((wt@x)^T (wp@x)) applied to (wv@x), all 1x1 convs.

Sharding: 8 cores = 4 samples x 2 halves of the N=4096 pixel axis.  Each core
computes both CMSA branches and the fused output for its 2048 columns,
flash-attention style (full 4096x4096 attention rows never materialized in
HBM).  Attention is computed in the "transposed" orientation LT[m, n] so that
softmax needs no PE transposes: exp is taken without max-subtraction (logits
are bounded ~|15| for this model scale, safe in f32), the denominator is a
ones-matmul over partitions, and the value bias bv is folded into an effective
output bias bo_eff = wo@bv + bo using softmax row-sum normalization.

All matmuls run as float32r (full PE rate, ~1e-4 relative error).
"""

import numpy as np

import concourse.bass as bass
import concourse.tile as tile
import concourse.mybir as mybir
from concourse import bacc
from concourse.bass_utils import run_bass_kernel_spmd

F32 = mybir.dt.float32
F32R = mybir.dt.float32r
F16 = mybir.dt.float16
AF = mybir.ActivationFunctionType
ALU = mybir.AluOpType

B = 4
H = W = 64
N = H * W            # 4096
NC = N // 2          # columns per core
A = 256
C_MM = 520
NB = 512             # psum column block
NSB = NC // NB       # 4 blocks per core chunk
MT = N // 128        # 32 m-tiles
KI = 5               # k-tiles for C=520 (4x128 + 8)
KL = 2               # k-tiles for C=256

_CACHE = {}


def _emit(nc, tc, T):
    """Emit the per-core program. T maps dram tensor names -> APs."""
    ones_f32 = None

    # ---- pools ---------------------------------------------------------
    # left stack: whole-kernel consts | theta/phi/VT (img then lang, tag-shared)
    pL1 = tc.alloc_tile_pool(name="consts", bufs=1, side="left")
    pL2 = tc.alloc_tile_pool(name="abc", bufs=1, side="left")
    # right stack: R1 mm23+spc (to end of lang) | R2 mm01+sp+img qkv w (img
    # qkv only) | R3 working set (attention + tails)
    pR1 = tc.alloc_tile_pool(name="mm23", bufs=1, side="right")
    pR2 = tc.alloc_tile_pool(name="mm01", bufs=1, side="right")
    pps = tc.alloc_tile_pool(name="ps", bufs=1, space="PSUM")

    # ---- consts --------------------------------------------------------
    ones32 = pL1.tile([128, 1], F32, tag="ones32")
    nc.vector.memset(ones32, 1.0)
    ones_r = pL1.tile([128, 1], F32R, tag="ones")
    nc.scalar.copy(out=ones_r, in_=ones32)
    bias_t = {}
    for nm in ("img_bt2", "img_bp2", "lang_bt2", "lang_bp2"):
        t = pL1.tile([128, 2], F32, tag=nm, name=nm)
        nc.sync.dma_start(out=t, in_=T[nm])
        bias_t[nm] = t
    part_out = pL1.tile([128, 2, NC], F16, tag="part_out")

    # ---- big inputs ----------------------------------------------------
    # Load order matters for PE warmup: img qkv weights and the spatial rows
    # first (every qkv psum chain ends on them), then mm in column chunks
    # breadth-first so the first qkv tiles can start after ~1/4 of the load.
    imgw = {}
    CS = N // 4
    # mm lives as [128, CS] chunk tiles so DMA->compute deps are exact
    mm_cs = [[None] * 4 for _ in range(4)]   # [k][cs]
    for k in (2, 3):
        for cs in range(4):
            mm_cs[k][cs] = pR1.tile([128, CS], F32R, tag=f"mm{k}c{cs}",
                                    name=f"mm{k}c{cs}")
    for k in (0, 1):
        for cs in range(4):
            mm_cs[k][cs] = pR2.tile([128, CS], F32R, tag=f"mm{k}c{cs}",
                                    name=f"mm{k}c{cs}")
    sp_sb = pR2.tile([8, N], F32R, tag="sp")
    for nm in ("img_wtT", "img_wpT", "img_wvT"):
        imgw[nm] = pR2.tile([128, KI, A], F32R, tag=nm, name=nm)

    def mm_cs_dma(cs):
        for k in range(4):
            nc.sync.dma_start(
                out=mm_cs[k][cs],
                in_=T["mm"][k * 128:(k + 1) * 128, cs * CS:(cs + 1) * CS].bitcast(F32R))

    nc.sync.dma_start(out=imgw["img_wtT"], in_=T["img_wtT"].bitcast(F32R))
    mm_cs_dma(0)
    nc.sync.dma_start(out=sp_sb, in_=T["mm"][512:520, :].bitcast(F32R))
    mm_cs_dma(1)
    nc.sync.dma_start(out=imgw["img_wpT"], in_=T["img_wpT"].bitcast(F32R))
    mm_cs_dma(2)
    nc.sync.dma_start(out=imgw["img_wvT"], in_=T["img_wvT"].bitcast(F32R))
    mm_cs_dma(3)

    def mm_ktile(k, cols):
        """[k-partitions, cols] slice of the mm operand for k-tile k.
        cols must lie within one CS-sized chunk for k < 4."""
        if k == 4:
            return sp_sb[:, cols]
        cs, lo, hi = cols.start // CS, cols.start % CS, None
        assert cols.stop - cols.start <= CS and cols.stop <= (cs + 1) * CS
        return mm_cs[k][cs][:, lo:lo + (cols.stop - cols.start)]

    def qkv_phase(branch, theta, phi, vt, wt, wp, wv, ks, bt2, bp2):
        """Computes theta [128,2,NC], phi [128,2,N], vt [128,MT,A] for one
        branch. ks = list of (ktile_idx, partitions)."""
        nk = len(ks)
        for a2 in range(2):
            asl = slice(a2 * 128, (a2 + 1) * 128)
            for ns in range(NSB):
                csl = slice(ns * NB, (ns + 1) * NB)
                q_ps = pps.tile([128, NB], F32, tag="blk", bufs=4, name="q_ps")
                for i, (k, kp) in enumerate(ks):
                    nc.tensor.matmul(q_ps, lhsT=wt[:kp, i, asl],
                                     rhs=mm_ktile(k, csl),
                                     start=(i == 0), stop=(i == nk - 1))
                nc.vector.tensor_scalar(out=theta[:, a2, csl], in0=q_ps,
                                        scalar1=bt2[:, a2:a2 + 1], scalar2=None,
                                        op0=ALU.add)
            for ns in range(N // NB):
                csl = slice(ns * NB, (ns + 1) * NB)
                q_ps = pps.tile([128, NB], F32, tag="blk", bufs=4, name="q_ps")
                for i, (k, kp) in enumerate(ks):
                    nc.tensor.matmul(q_ps, lhsT=wp[:kp, i, asl],
                                     rhs=mm_ktile(k, csl),
                                     start=(i == 0), stop=(i == nk - 1))
                nc.vector.tensor_scalar(out=phi[:, a2, csl], in0=q_ps,
                                        scalar1=bp2[:, a2:a2 + 1], scalar2=None,
                                        op0=ALU.add)
        for m in range(MT):
            msl = slice(m * 128, (m + 1) * 128)
            v_ps = pps.tile([128, A], F32, tag="blk", bufs=4, name="v_ps")
            for i, (k, kp) in enumerate(ks):
                nc.tensor.matmul(v_ps, lhsT=mm_ktile(k, msl)[:kp, :],
                                 rhs=wv[:kp, i, :],
                                 start=(i == 0), stop=(i == nk - 1))
            nc.vector.tensor_copy(out=vt[:, m, :], in_=v_ps)

    def attn_phase(branch, theta, phi, vt, pools):
        """Flash attention + folded G conv.  The per-nb tail (rs -> rcp -> bc
        -> G -> normalize -> output) is emitted inside the NEXT nb's m-loop so
        the in-order PE never stalls on the normalization chain."""
        woT = pools["woT_" + branch]
        fusT = pools["fus_sp2"]
        pR3 = pools["pR3"]
        pend = None

        def emit_G(pend):
            g_ps = pps.tile([128, 2, NB], F32, tag="big2", bufs=2, name="g_ps")
            for q2 in range(2):
                qsl = slice(q2 * 128, (q2 + 1) * 128)
                for k2 in range(2):
                    nc.tensor.matmul(g_ps[:, q2, :], lhsT=woT[:, k2, qsl],
                                     rhs=pend["att_sb"][:, k2, :],
                                     start=(k2 == 0), stop=(k2 == 1))
            pend["g_ps"] = g_ps

        def emit_out(pend):
            csl = slice(pend["nb"] * NB, (pend["nb"] + 1) * NB)
            for q2 in range(2):
                t1 = pR3.tile([128, NB], F16, tag="t1", bufs=4, name="t1")
                nc.vector.tensor_tensor(out=t1, in0=pend["g_ps"][:, q2, :],
                                        in1=pend["bc"], op=ALU.mult)
                if branch == "img":
                    nc.vector.tensor_tensor(out=part_out[:, q2, csl], in0=t1,
                                            in1=fusT[:, q2, csl], op=ALU.add)
                else:
                    out_t = pR3.tile([128, NB], F16, tag="out_t", bufs=2, name="out_t")
                    nc.vector.tensor_tensor(out=out_t, in0=t1,
                                            in1=part_out[:, q2, csl], op=ALU.add)
                    nc.sync.dma_start(
                        out=T["out"][q2 * 128:(q2 + 1) * 128, csl], in_=out_t)

        for nb in range(NSB):
            csl = slice(nb * NB, (nb + 1) * NB)
            att_ps = pps.tile([128, 2, NB], F32, tag="big2", bufs=2, name="att_ps")
            acc = pR3.tile([128, NB], F32R, tag="acc", bufs=2, name="acc")

            def pv(m_idx, p_tile):
                for a2 in range(2):
                    nc.tensor.matmul(att_ps[:, a2, :],
                                     lhsT=vt[:, m_idx, a2 * 128:(a2 + 1) * 128],
                                     rhs=p_tile,
                                     start=(m_idx == 0), stop=(m_idx == MT - 1))

            # denominator accumulates on two engines: DVE (m<24) and Pool
            # (m>=24) so neither becomes the bottleneck; rs sums both.
            acc2 = pR3.tile([128, NB], F32R, tag="acc2", bufs=2, name="acc2")
            prev_p = None
            for m in range(MT):
                msl = slice(m * 128, (m + 1) * 128)
                lt_ps = pps.tile([128, NB], F32, tag="blk", bufs=4, name="lt_ps")
                for ka in range(2):
                    nc.tensor.matmul(lt_ps, lhsT=phi[:, ka, msl],
                                     rhs=theta[:, ka, csl],
                                     start=(ka == 0), stop=(ka == 1))
                if m == 2 and pend is not None:
                    emit_G(pend)
                if prev_p is not None:
                    pv(m - 1, prev_p)
                if m == 5 and pend is not None:
                    emit_out(pend)
                    pend = None
                p_sb = pR3.tile([128, NB], F32R, tag="p", bufs=5, name="p_sb")
                nc.scalar.activation(out=p_sb, in_=lt_ps, func=AF.Exp)
                if m == 0:
                    nc.vector.tensor_copy(out=acc, in_=p_sb)
                elif m == 24:
                    nc.gpsimd.tensor_copy(out=acc2, in_=p_sb)
                elif m > 24:
                    nc.gpsimd.tensor_add(out=acc2, in0=acc2, in1=p_sb)
                else:
                    nc.vector.tensor_add(out=acc, in0=acc, in1=p_sb)
                prev_p = p_sb
            pv(MT - 1, prev_p)
            # softmax denominator -> reciprocal -> broadcast (tail, pipelined)
            rs_ps = pps.tile([128, NB], F32, tag="blk", bufs=4, name="rs_ps")
            nc.tensor.matmul(rs_ps[0:1, :], lhsT=ones_r, rhs=acc,
                             start=True, stop=False)
            nc.tensor.matmul(rs_ps[0:1, :], lhsT=ones_r, rhs=acc2,
                             start=False, stop=True)
            att_sb = pR3.tile([128, 2, NB], F32R, tag="att_sb", bufs=3, name="att_sb")
            for a2 in range(2):
                nc.scalar.copy(out=att_sb[:, a2, :], in_=att_ps[:, a2, :])
            rcp = pR3.tile([1, NB], F32, tag="rcp", bufs=2, name="rcp")
            nc.vector.reciprocal(out=rcp, in_=rs_ps[0:1, :])
            bc = pR3.tile([128, NB], F32, tag="bc", bufs=2, name="bc")
            nc.gpsimd.partition_broadcast(bc, rcp)
            pend = {"nb": nb, "att_sb": att_sb, "bc": bc}
        emit_G(pend)
        emit_out(pend)

    # ---- img qkv -------------------------------------------------------
    theta = pL2.tile([128, 2, NC], F32R, tag="theta", name="theta_i")
    phi = pL2.tile([128, 2, N], F32R, tag="phi", name="phi_i")
    vt = pL2.tile([128, MT, A], F32R, tag="vt", name="vt_i")
    ks_img = [(0, 128), (1, 128), (2, 128), (3, 128), (4, 8)]
    qkv_phase("img", theta, phi, vt, imgw["img_wtT"], imgw["img_wpT"],
              imgw["img_wvT"], ks_img, bias_t["img_bt2"], bias_t["img_bp2"])
    pR2.release()

    # ---- working pool (attention + tails) ------------------------------
    pR3 = tc.alloc_tile_pool(name="work", bufs=1, side="right")
    pools = {"pR3": pR3}
    for nm, kt in (("woT_img", 2), ("woT_lang", 2),
                   ("lang_wtT", KL), ("lang_wpT", KL), ("lang_wvT", KL)):
        dnm = {"woT_img": "GimgT", "woT_lang": "GlangT"}.get(nm, nm)
        t = pR3.tile([128, kt, A], F32R, tag=nm, name=nm)
        nc.sync.dma_start(out=t, in_=T[dnm].bitcast(F32R))
        pools[nm] = t
    fus_sp2 = pR3.tile([128, 2, NC], F16, tag="fus_sp2", name="fus_sp2")
    nc.sync.dma_start(out=fus_sp2, in_=T["fus_sp2"])
    pools["fus_sp2"] = fus_sp2

    # ---- img attention + partial fusion --------------------------------
    attn_phase("img", theta, phi, vt, pools)

    # ---- lang qkv ------------------------------------------------------
    theta_l = pL2.tile([128, 2, NC], F32R, tag="theta", name="theta_l")
    phi_l = pL2.tile([128, 2, N], F32R, tag="phi", name="phi_l")
    vt_l = pL2.tile([128, MT, A], F32R, tag="vt", name="vt_l")
    ks_lang = [(2, 128), (3, 128)]
    qkv_phase("lang", theta_l, phi_l, vt_l, pools["lang_wtT"],
              pools["lang_wpT"], pools["lang_wvT"], ks_lang,
              bias_t["lang_bt2"], bias_t["lang_bp2"])

    # ---- lang attention + final output ---------------------------------
    attn_phase("lang", theta_l, phi_l, vt_l, pools)

    pR3.release()
    pR1.release()
    pL2.release()
    pL1.release()
    pps.release()


def _build(repeat=1):
    nc = bacc.Bacc("TRN2", target_bir_lowering=False, debug=False, num_devices=8)
    T = {}
    T["mm"] = nc.dram_tensor("mm", [C_MM, N], F32, kind="ExternalInput").ap()
    for nm in ("img_wtT", "img_wpT", "img_wvT"):
        T[nm] = nc.dram_tensor(nm, [128, KI, A], F32, kind="ExternalInput").ap()
    for nm in ("lang_wtT", "lang_wpT", "lang_wvT"):
        T[nm] = nc.dram_tensor(nm, [128, KL, A], F32, kind="ExternalInput").ap()
    for nm in ("GimgT", "GlangT"):
        T[nm] = nc.dram_tensor(nm, [128, 2, A], F32, kind="ExternalInput").ap()
    T["fus_sp2"] = nc.dram_tensor("fus_sp2", [128, 2, NC], F16,
                                  kind="ExternalInput").ap()
    for nm in ("img_bt2", "img_bp2", "lang_bt2", "lang_bp2"):
        T[nm] = nc.dram_tensor(nm, [128, 2], F32, kind="ExternalInput").ap()
    T["out"] = nc.dram_tensor("out", [A, NC], F16, kind="ExternalOutput").ap()

    with tile.TileContext(nc) as tc:
        for _ in range(repeat):
            _emit(nc, tc, T)
    nc.compile()
    return nc


def _spatial():
    gy, gx = np.meshgrid(np.linspace(0, 1, H, dtype=np.float32),
                         np.linspace(0, 1, W, dtype=np.float32), indexing="ij")
    feats = [gx, gy, 1.0 - gx, 1.0 - gy] + [(gx + gy) * 0.5] * 4
    return np.stack(feats[:8], axis=0).reshape(8, N).astype(np.float32)


def _pack_kT(wT, kt):
    """[C, A] (pre-transposed weight) -> [128, kt, A] partition-tiled."""
    out = np.zeros((128, kt, wT.shape[1]), np.float32)
    for k in range(kt):
        rows = wT[k * 128:min((k + 1) * 128, wT.shape[0])]
        out[:rows.shape[0], k] = rows
    return out


def _bias2(b):
    return np.ascontiguousarray(b.reshape(2, 128).T)


def _in_maps(inputs):
    f = lambda k: np.asarray(inputs[k], np.float32)
    images, flows = f("images"), f("flows")
    sp = _spatial()

    # fold wo through the fusion conv; fold bv/bo/spatial/fus_b into one field
    G_img = f("fus_w")[:, 0:256] @ f("img_wo")
    G_lang = f("fus_w")[:, 256:512] @ f("lang_wo")
    bo_eff_img = f("img_wo") @ f("img_bv") + f("img_bo")
    bo_eff_lang = f("lang_wo") @ f("lang_bv") + f("lang_bo")
    fus_b_eff = (f("fus_b") + f("fus_w")[:, 0:256] @ bo_eff_img
                 + f("fus_w")[:, 256:512] @ bo_eff_lang)
    fus_sp_full = f("fus_w")[:, 512:520] @ sp + fus_b_eff[:, None]  # [256, N]

    base = {
        "img_wtT": _pack_kT(f("img_wt").T, KI),
        "img_wpT": _pack_kT(f("img_wp").T, KI),
        "img_wvT": _pack_kT(f("img_wv").T, KI),
        "lang_wtT": _pack_kT(f("lang_wt").T, KL),
        "lang_wpT": _pack_kT(f("lang_wp").T, KL),
        "lang_wvT": _pack_kT(f("lang_wv").T, KL),
        "GimgT": _pack_kT(G_img.T, 2),
        "GlangT": _pack_kT(G_lang.T, 2),
        "img_bt2": _bias2(f("img_bt")),
        "img_bp2": _bias2(f("img_bp")),
        "lang_bt2": _bias2(f("lang_bt")),
        "lang_bp2": _bias2(f("lang_bp")),
    }

    in_maps = []
    for c in range(8):
        b, half = c // 2, c % 2
        mm = np.concatenate(
            [images[b].reshape(256, N), flows[b].reshape(256, N), sp], axis=0)
        if half:
            mm = np.roll(mm, -NC, axis=1)
        fsp = np.roll(fus_sp_full, -half * NC, axis=1)[:, :NC]
        fsp2 = np.ascontiguousarray(
            fsp.reshape(2, 128, NC).transpose(1, 0, 2).astype(np.float16))
        in_maps.append({**base, "mm": np.ascontiguousarray(mm), "fus_sp2": fsp2})
    return in_maps


def kernel(**inputs):
    if "nc" not in _CACHE:
        _CACHE["nc"] = _build()
    nc = _CACHE["nc"]
    in_maps = _in_maps(inputs)
    res = run_bass_kernel_spmd(nc, in_maps, list(range(8)))
    out = np.empty((B, A, N), np.float32)
    for c in range(8):
        b, half = c // 2, c % 2
        out[b][:, half * NC:(half + 1) * NC] = res.results[c]["out"].astype(np.float32)
    return out.reshape(B, A, H, W)



# revision 2
# speedup vs baseline: 1.0083x; 1.0083x over previous
"""CMSA (cross-modal self-attention) model on 8 Trainium2 NeuronCores.

Model (B=4, C=256, H=W=64, N=4096, A=256):
  spatial = fixed 8-channel coordinate features            [B, 8, H, W]
  mm   = concat(images, flows, spatial)                    [B, 520, H, W]
  img_feat  = CMSA(mm,   img_w*)                           [B, 256, H, W]
  lang_feat = CMSA(flows, lang_w*)                         [B, 256, H, W]
  out = conv1x1(concat(img_feat, lang_feat, spatial), fus) [B, 256, H, W]
where CMSA(x) = wo @ softmax((wt@x)^T (wp@x)) applied to (wv@x), all 1x1 convs.

Sharding: 8 cores = 4 samples x 2 halves of the N=4096 pixel axis.  Each core
computes both CMSA branches and the fused output for its 2048 columns,
flash-attention style (full 4096x4096 attention rows never materialized in
HBM).  Attention is computed in the "transposed" orientation LT[m, n] so that
softmax needs no PE transposes: exp is taken without max-subtraction (logits
are bounded ~|15| for this model scale, safe in f32), the denominator is a
ones-matmul over partitions, and the value bias bv is folded into an effective
output bias bo_eff = wo@bv + bo using softmax row-sum normalization.

All matmuls run as float32r (full PE rate, ~1e-4 relative error).
"""

import numpy as np

import concourse.bass as bass
import concourse.tile as tile
import concourse.mybir as mybir
from concourse import bacc
from concourse.bass_utils import run_bass_kernel_spmd

F32 = mybir.dt.float32
F32R = mybir.dt.float32r
F16 = mybir.dt.float16
BF16 = mybir.dt.bfloat16
AF = mybir.ActivationFunctionType
ALU = mybir.AluOpType

B = 4
H = W = 64
N = H * W            # 4096
NC = N // 2          # columns per core
A = 256
C_MM = 520
NB = 512             # psum column block
NSB = NC // NB       # 4 blocks per core chunk
MT = N // 128        # 32 m-tiles
KI = 5               # k-tiles for C=520 (4x128 + 8)
KL = 2               # k-tiles for C=256

_CACHE = {}


def _emit(nc, tc, T):
    """Emit the per-core program. T maps dram tensor names -> APs."""
    ones_f32 = None

    # ---- pools ---------------------------------------------------------
    # left stack: whole-kernel consts | theta/phi/VT (img then lang, tag-shared)
    pL1 = tc.alloc_tile_pool(name="consts", bufs=1, side="left")
    pL2 = tc.alloc_tile_pool(name="abc", bufs=1, side="left")
    # right stack: R1 mm23+spc (to end of lang) | R2 mm01+sp+img qkv w (img
    # qkv only) | R3 working set (attention + tails)
    pR1 = tc.alloc_tile_pool(name="mm23", bufs=1, side="right")
    pR2 = tc.alloc_tile_pool(name="mm01", bufs=1, side="right")
    pps = tc.alloc_tile_pool(name="ps", bufs=1, space="PSUM")

    # ---- consts --------------------------------------------------------
    ones32 = pL1.tile([128, 1], F32, tag="ones32")
    nc.vector.memset(ones32, 1.0)
    ones_r = pL1.tile([128, 1], BF16, tag="ones")
    nc.scalar.copy(out=ones_r, in_=ones32)
    bias_t = {}
    for nm in ("img_bt2", "img_bp2", "lang_bt2", "lang_bp2"):
        t = pL1.tile([128, 2], F32, tag=nm, name=nm)
        nc.sync.dma_start(out=t, in_=T[nm])
        bias_t[nm] = t
    part_out = pL1.tile([128, 2, NC], F16, tag="part_out")

    # ---- big inputs ----------------------------------------------------
    # Load order matters for PE warmup: img qkv weights and the spatial rows
    # first (every qkv psum chain ends on them), then mm in column chunks
    # breadth-first so the first qkv tiles can start after ~1/4 of the load.
    imgw = {}
    CS = N // 4
    # mm lives as [128, CS] chunk tiles so DMA->compute deps are exact
    mm_cs = [[None] * 4 for _ in range(4)]   # [k][cs]
    for k in (2, 3):
        for cs in range(4):
            mm_cs[k][cs] = pR1.tile([128, CS], BF16, tag=f"mm{k}c{cs}",
                                    name=f"mm{k}c{cs}")
    for k in (0, 1):
        for cs in range(4):
            mm_cs[k][cs] = pR2.tile([128, CS], BF16, tag=f"mm{k}c{cs}",
                                    name=f"mm{k}c{cs}")
    sp_sb = pR2.tile([8, N], BF16, tag="sp")
    for nm in ("img_wtT", "img_wpT", "img_wvT"):
        imgw[nm] = pR2.tile([128, KI, A], BF16, tag=nm, name=nm)

    def mm_cs_dma(cs):
        for k in range(4):
            nc.sync.dma_start(
                out=mm_cs[k][cs],
                in_=T["mm"][k * 128:(k + 1) * 128, cs * CS:(cs + 1) * CS])

    nc.sync.dma_start(out=imgw["img_wtT"], in_=T["img_wtT"])
    mm_cs_dma(0)
    nc.sync.dma_start(out=sp_sb, in_=T["mm"][512:520, :])
    mm_cs_dma(1)
    nc.sync.dma_start(out=imgw["img_wpT"], in_=T["img_wpT"])
    mm_cs_dma(2)
    nc.sync.dma_start(out=imgw["img_wvT"], in_=T["img_wvT"])
    mm_cs_dma(3)

    def mm_ktile(k, cols):
        """[k-partitions, cols] slice of the mm operand for k-tile k.
        cols must lie within one CS-sized chunk for k < 4."""
        if k == 4:
            return sp_sb[:, cols]
        cs, lo, hi = cols.start // CS, cols.start % CS, None
        assert cols.stop - cols.start <= CS and cols.stop <= (cs + 1) * CS
        return mm_cs[k][cs][:, lo:lo + (cols.stop - cols.start)]

    def qkv_phase(branch, theta, phi, vt, wt, wp, wv, ks, bt2, bp2):
        """Computes theta [128,2,NC], phi [128,2,N], vt [128,MT,A] for one
        branch. ks = list of (ktile_idx, partitions)."""
        nk = len(ks)
        for a2 in range(2):
            asl = slice(a2 * 128, (a2 + 1) * 128)
            for ns in range(NSB):
                csl = slice(ns * NB, (ns + 1) * NB)
                q_ps = pps.tile([128, NB], F32, tag="blk", bufs=4, name="q_ps")
                for i, (k, kp) in enumerate(ks):
                    nc.tensor.matmul(q_ps, lhsT=wt[:kp, i, asl],
                                     rhs=mm_ktile(k, csl),
                                     start=(i == 0), stop=(i == nk - 1))
                nc.vector.tensor_scalar(out=theta[:, a2, csl], in0=q_ps,
                                        scalar1=bt2[:, a2:a2 + 1], scalar2=None,
                                        op0=ALU.add)
            for ns in range(N // NB):
                csl = slice(ns * NB, (ns + 1) * NB)
                q_ps = pps.tile([128, NB], F32, tag="blk", bufs=4, name="q_ps")
                for i, (k, kp) in enumerate(ks):
                    nc.tensor.matmul(q_ps, lhsT=wp[:kp, i, asl],
                                     rhs=mm_ktile(k, csl),
                                     start=(i == 0), stop=(i == nk - 1))
                nc.vector.tensor_scalar(out=phi[:, a2, csl], in0=q_ps,
                                        scalar1=bp2[:, a2:a2 + 1], scalar2=None,
                                        op0=ALU.add)
        for m in range(MT):
            msl = slice(m * 128, (m + 1) * 128)
            v_ps = pps.tile([128, A], F32, tag="blk", bufs=4, name="v_ps")
            for i, (k, kp) in enumerate(ks):
                nc.tensor.matmul(v_ps, lhsT=mm_ktile(k, msl)[:kp, :],
                                 rhs=wv[:kp, i, :],
                                 start=(i == 0), stop=(i == nk - 1))
            nc.vector.tensor_copy(out=vt[:, m, :], in_=v_ps)

    def attn_phase(branch, theta, phi, vt, pools):
        """Flash attention + folded G conv.  The per-nb tail (rs -> rcp -> bc
        -> G -> normalize -> output) is emitted inside the NEXT nb's m-loop so
        the in-order PE never stalls on the normalization chain."""
        woT = pools["woT_" + branch]
        fusT = pools["fus_sp2"]
        pR3 = pools["pR3"]
        pend = None

        def emit_G(pend):
            g_ps = pps.tile([128, 2, NB], F32, tag="big2", bufs=2, name="g_ps")
            for q2 in range(2):
                qsl = slice(q2 * 128, (q2 + 1) * 128)
                for k2 in range(2):
                    nc.tensor.matmul(g_ps[:, q2, :], lhsT=woT[:, k2, qsl],
                                     rhs=pend["att_sb"][:, k2, :],
                                     start=(k2 == 0), stop=(k2 == 1))
            pend["g_ps"] = g_ps

        def emit_out(pend):
            csl = slice(pend["nb"] * NB, (pend["nb"] + 1) * NB)
            for q2 in range(2):
                t1 = pR3.tile([128, NB], F16, tag="t1", bufs=4, name="t1")
                nc.vector.tensor_tensor(out=t1, in0=pend["g_ps"][:, q2, :],
                                        in1=pend["bc"], op=ALU.mult)
                if branch == "img":
                    nc.vector.tensor_tensor(out=part_out[:, q2, csl], in0=t1,
                                            in1=fusT[:, q2, csl], op=ALU.add)
                else:
                    out_t = pR3.tile([128, NB], F16, tag="out_t", bufs=2, name="out_t")
                    nc.vector.tensor_tensor(out=out_t, in0=t1,
                                            in1=part_out[:, q2, csl], op=ALU.add)
                    nc.sync.dma_start(
                        out=T["out"][q2 * 128:(q2 + 1) * 128, csl], in_=out_t)

        for nb in range(NSB):
            csl = slice(nb * NB, (nb + 1) * NB)
            att_ps = pps.tile([128, 2, NB], F32, tag="big2", bufs=2, name="att_ps")
            acc = pR3.tile([128, NB], BF16, tag="acc", bufs=2, name="acc")

            def pv(m_idx, p_tile):
                for a2 in range(2):
                    nc.tensor.matmul(att_ps[:, a2, :],
                                     lhsT=vt[:, m_idx, a2 * 128:(a2 + 1) * 128],
                                     rhs=p_tile,
                                     start=(m_idx == 0), stop=(m_idx == MT - 1))

            # denominator accumulates on two engines: DVE (m<24) and Pool
            # (m>=24) so neither becomes the bottleneck; rs sums both.
            acc2 = pR3.tile([128, NB], BF16, tag="acc2", bufs=2, name="acc2")
            prev_p = None
            for m in range(MT):
                msl = slice(m * 128, (m + 1) * 128)
                lt_ps = pps.tile([128, NB], F32, tag="blk", bufs=4, name="lt_ps")
                for ka in range(2):
                    nc.tensor.matmul(lt_ps, lhsT=phi[:, ka, msl],
                                     rhs=theta[:, ka, csl],
                                     start=(ka == 0), stop=(ka == 1))
                if m == 2 and pend is not None:
                    emit_G(pend)
                if prev_p is not None:
                    pv(m - 1, prev_p)
                if m == 5 and pend is not None:
                    emit_out(pend)
                    pend = None
                p_sb = pR3.tile([128, NB], BF16, tag="p", bufs=5, name="p_sb")
                nc.scalar.activation(out=p_sb, in_=lt_ps, func=AF.Exp)
                if m == 0:
                    nc.vector.tensor_copy(out=acc, in_=p_sb)
                elif m == 24:
                    nc.gpsimd.tensor_copy(out=acc2, in_=p_sb)
                elif m > 24:
                    nc.gpsimd.tensor_add(out=acc2, in0=acc2, in1=p_sb)
                else:
                    nc.vector.tensor_add(out=acc, in0=acc, in1=p_sb)
                prev_p = p_sb
            pv(MT - 1, prev_p)
            # softmax denominator -> reciprocal -> broadcast (tail, pipelined)
            rs_ps = pps.tile([128, NB], F32, tag="blk", bufs=4, name="rs_ps")
            nc.tensor.matmul(rs_ps[0:1, :], lhsT=ones_r, rhs=acc,
                             start=True, stop=False)
            nc.tensor.matmul(rs_ps[0:1, :], lhsT=ones_r, rhs=acc2,
                             start=False, stop=True)
            att_sb = pR3.tile([128, 2, NB], BF16, tag="att_sb", bufs=3, name="att_sb")
            for a2 in range(2):
                nc.scalar.copy(out=att_sb[:, a2, :], in_=att_ps[:, a2, :])
            rcp = pR3.tile([1, NB], F32, tag="rcp", bufs=2, name="rcp")
            nc.vector.reciprocal(out=rcp, in_=rs_ps[0:1, :])
            bc = pR3.tile([128, NB], F32, tag="bc", bufs=2, name="bc")
            nc.gpsimd.partition_broadcast(bc, rcp)
            pend = {"nb": nb, "att_sb": att_sb, "bc": bc}
        emit_G(pend)
        emit_out(pend)

    # ---- img qkv -------------------------------------------------------
    theta = pL2.tile([128, 2, NC], BF16, tag="theta", name="theta_i")
    phi = pL2.tile([128, 2, N], BF16, tag="phi", name="phi_i")
    vt = pL2.tile([128, MT, A], BF16, tag="vt", name="vt_i")
    ks_img = [(0, 128), (1, 128), (2, 128), (3, 128), (4, 8)]
    qkv_phase("img", theta, phi, vt, imgw["img_wtT"], imgw["img_wpT"],
              imgw["img_wvT"], ks_img, bias_t["img_bt2"], bias_t["img_bp2"])
    pR2.release()

    # ---- working pool (attention + tails) ------------------------------
    pR3 = tc.alloc_tile_pool(name="work", bufs=1, side="right")
    pools = {"pR3": pR3}
    for nm, kt in (("woT_img", 2), ("woT_lang", 2),
                   ("lang_wtT", KL), ("lang_wpT", KL), ("lang_wvT", KL)):
        dnm = {"woT_img": "GimgT", "woT_lang": "GlangT"}.get(nm, nm)
        t = pR3.tile([128, kt, A], BF16, tag=nm, name=nm)
        nc.sync.dma_start(out=t, in_=T[dnm])
        pools[nm] = t
    fus_sp2 = pR3.tile([128, 2, NC], F16, tag="fus_sp2", name="fus_sp2")
    nc.sync.dma_start(out=fus_sp2, in_=T["fus_sp2"])
    pools["fus_sp2"] = fus_sp2

    # ---- img attention + partial fusion --------------------------------
    attn_phase("img", theta, phi, vt, pools)

    # ---- lang qkv ------------------------------------------------------
    theta_l = pL2.tile([128, 2, NC], BF16, tag="theta", name="theta_l")
    phi_l = pL2.tile([128, 2, N], BF16, tag="phi", name="phi_l")
    vt_l = pL2.tile([128, MT, A], BF16, tag="vt", name="vt_l")
    ks_lang = [(2, 128), (3, 128)]
    qkv_phase("lang", theta_l, phi_l, vt_l, pools["lang_wtT"],
              pools["lang_wpT"], pools["lang_wvT"], ks_lang,
              bias_t["lang_bt2"], bias_t["lang_bp2"])

    # ---- lang attention + final output ---------------------------------
    attn_phase("lang", theta_l, phi_l, vt_l, pools)

    pR3.release()
    pR1.release()
    pL2.release()
    pL1.release()
    pps.release()


def _build(repeat=1):
    nc = bacc.Bacc("TRN2", target_bir_lowering=False, debug=False, num_devices=8)
    T = {}
    T["mm"] = nc.dram_tensor("mm", [C_MM, N], BF16, kind="ExternalInput").ap()
    for nm in ("img_wtT", "img_wpT", "img_wvT"):
        T[nm] = nc.dram_tensor(nm, [128, KI, A], BF16, kind="ExternalInput").ap()
    for nm in ("lang_wtT", "lang_wpT", "lang_wvT"):
        T[nm] = nc.dram_tensor(nm, [128, KL, A], BF16, kind="ExternalInput").ap()
    for nm in ("GimgT", "GlangT"):
        T[nm] = nc.dram_tensor(nm, [128, 2, A], BF16, kind="ExternalInput").ap()
    T["fus_sp2"] = nc.dram_tensor("fus_sp2", [128, 2, NC], F16,
                                  kind="ExternalInput").ap()
    for nm in ("img_bt2", "img_bp2", "lang_bt2", "lang_bp2"):
        T[nm] = nc.dram_tensor(nm, [128, 2], F32, kind="ExternalInput").ap()
    T["out"] = nc.dram_tensor("out", [A, NC], F16, kind="ExternalOutput").ap()

    with tile.TileContext(nc) as tc:
        for _ in range(repeat):
            _emit(nc, tc, T)
    nc.compile()
    return nc


def _spatial():
    gy, gx = np.meshgrid(np.linspace(0, 1, H, dtype=np.float32),
                         np.linspace(0, 1, W, dtype=np.float32), indexing="ij")
    feats = [gx, gy, 1.0 - gx, 1.0 - gy] + [(gx + gy) * 0.5] * 4
    return np.stack(feats[:8], axis=0).reshape(8, N).astype(np.float32)


BF16_NP = mybir.dt.np(BF16)


def _pack_kT(wT, kt):
    """[C, A] (pre-transposed weight) -> [128, kt, A] partition-tiled."""
    out = np.zeros((128, kt, wT.shape[1]), np.float32)
    for k in range(kt):
        rows = wT[k * 128:min((k + 1) * 128, wT.shape[0])]
        out[:rows.shape[0], k] = rows
    return out.astype(BF16_NP)


def _bias2(b):
    return np.ascontiguousarray(b.reshape(2, 128).T)


def _in_maps(inputs):
    f = lambda k: np.asarray(inputs[k], np.float32)
    images, flows = f("images"), f("flows")
    sp = _spatial()

    # fold wo through the fusion conv; fold bv/bo/spatial/fus_b into one field
    G_img = f("fus_w")[:, 0:256] @ f("img_wo")
    G_lang = f("fus_w")[:, 256:512] @ f("lang_wo")
    bo_eff_img = f("img_wo") @ f("img_bv") + f("img_bo")
    bo_eff_lang = f("lang_wo") @ f("lang_bv") + f("lang_bo")
    fus_b_eff = (f("fus_b") + f("fus_w")[:, 0:256] @ bo_eff_img
                 + f("fus_w")[:, 256:512] @ bo_eff_lang)
    fus_sp_full = f("fus_w")[:, 512:520] @ sp + fus_b_eff[:, None]  # [256, N]

    base = {
        "img_wtT": _pack_kT(f("img_wt").T, KI),
        "img_wpT": _pack_kT(f("img_wp").T, KI),
        "img_wvT": _pack_kT(f("img_wv").T, KI),
        "lang_wtT": _pack_kT(f("lang_wt").T, KL),
        "lang_wpT": _pack_kT(f("lang_wp").T, KL),
        "lang_wvT": _pack_kT(f("lang_wv").T, KL),
        "GimgT": _pack_kT(G_img.T, 2),
        "GlangT": _pack_kT(G_lang.T, 2),
        "img_bt2": _bias2(f("img_bt")),
        "img_bp2": _bias2(f("img_bp")),
        "lang_bt2": _bias2(f("lang_bt")),
        "lang_bp2": _bias2(f("lang_bp")),
    }

    in_maps = []
    for c in range(8):
        b, half = c // 2, c % 2
        mm = np.concatenate(
            [images[b].reshape(256, N), flows[b].reshape(256, N), sp], axis=0)
        if half:
            mm = np.roll(mm, -NC, axis=1)
        fsp = np.roll(fus_sp_full, -half * NC, axis=1)[:, :NC]
        fsp2 = np.ascontiguousarray(
            fsp.reshape(2, 128, NC).transpose(1, 0, 2).astype(np.float16))
        in_maps.append({**base, "mm": np.ascontiguousarray(mm).astype(BF16_NP), "fus_sp2": fsp2})
    return in_maps


def kernel(**inputs):
    if "nc" not in _CACHE:
        _CACHE["nc"] = _build()
    nc = _CACHE["nc"]
    in_maps = _in_maps(inputs)
    res = run_bass_kernel_spmd(nc, in_maps, list(range(8)))
    out = np.empty((B, A, N), np.float32)
    for c in range(8):
        b, half = c // 2, c % 2
        out[b][:, half * NC:(half + 1) * NC] = res.results[c]["out"].astype(np.float32)
    return out.reshape(B, A, H, W)



# revision 13
# speedup vs baseline: 2.0256x; 2.0089x over previous
"""CMSA (cross-modal self-attention) model on 8 Trainium2 NeuronCores.

Model (B=4, C=256, H=W=64, N=4096, A=256):
  spatial = fixed 8-channel coordinate features            [B, 8, H, W]
  mm   = concat(images, flows, spatial)                    [B, 520, H, W]
  img_feat  = CMSA(mm,   img_w*)                           [B, 256, H, W]
  lang_feat = CMSA(flows, lang_w*)                         [B, 256, H, W]
  out = conv1x1(concat(img_feat, lang_feat, spatial), fus) [B, 256, H, W]
where CMSA(x) = wo @ softmax((wt@x)^T (wp@x)) applied to (wv@x), all 1x1 convs.

Sharding: 8 cores = 4 samples x 2 halves of the N=4096 pixel axis.  Each core
computes both CMSA branches and the fused output for its 2048 columns,
flash-attention style (full 4096x4096 attention rows never materialized in
HBM).  Attention is computed in the "transposed" orientation LT[m, n] so that
softmax needs no PE transposes: exp is taken without max-subtraction (logits
are bounded ~|15| for this model scale, safe in f32), the denominator is a
ones-matmul over partitions, and the value bias bv is folded into an effective
output bias bo_eff = wo@bv + bo using softmax row-sum normalization.

All matmuls run as float32r (full PE rate, ~1e-4 relative error).
"""

import os

import numpy as np

import concourse.bass as bass
import concourse.tile as tile
import concourse.mybir as mybir
from concourse import bacc
from concourse.bass_utils import run_bass_kernel_spmd

F32 = mybir.dt.float32
F32R = mybir.dt.float32r
F16 = mybir.dt.float16
AF = mybir.ActivationFunctionType
ALU = mybir.AluOpType

B = 4
H = W = 64
N = H * W            # 4096
NC = N // 2          # columns per core
A = 256
C_MM = 520
NB = 512             # psum column block
NSB = NC // NB       # 4 blocks per core chunk
MT = N // 128        # 32 m-tiles
KI = 5               # k-tiles for C=520 (4x128 + 8)
KL = 2               # k-tiles for C=256

_CACHE = {}


def _emit(nc, tc, T):
    """Emit the per-core program. T maps dram tensor names -> APs."""
    ones_f32 = None

    # ---- pools ---------------------------------------------------------
    # left stack: whole-kernel consts | theta/phi/VT (img then lang, tag-shared)
    pL1 = tc.alloc_tile_pool(name="consts", bufs=1, side="left")
    pL2 = tc.alloc_tile_pool(name="abc", bufs=1, side="left")
    # right stack: R1 mm23+spc (to end of lang) | R2 mm01+sp+img qkv w (img
    # qkv only) | R3 working set (attention + tails)
    pR1 = tc.alloc_tile_pool(name="mm23", bufs=1, side="right")
    pR2 = tc.alloc_tile_pool(name="mm01", bufs=1, side="right")
    pps = tc.alloc_tile_pool(name="ps", bufs=1, space="PSUM")

    # ---- consts --------------------------------------------------------
    ones32 = pL1.tile([128, 1], F32, tag="ones32")
    nc.vector.memset(ones32, 1.0)
    ones_r = pL1.tile([128, 1], F32R, tag="ones")
    nc.scalar.copy(out=ones_r, in_=ones32)
    bias_t = {}
    for nm in ("img_bt2", "img_bp2", "lang_bt2", "lang_bp2"):
        t = pL1.tile([128, 2], F32, tag=nm, name=nm)
        nc.sync.dma_start(out=t, in_=T[nm])
        bias_t[nm] = t
    part_out = pL1.tile([128, 2, NC], F16, tag="part_out")

    # ---- big inputs ----------------------------------------------------
    # Load order matters for PE warmup: img qkv weights and the spatial rows
    # first (every qkv psum chain ends on them), then mm in column chunks
    # breadth-first so the first qkv tiles can start after ~1/4 of the load.
    imgw = {}
    CS = N // 4
    # mm lives as [128, CS] chunk tiles so DMA->compute deps are exact
    mm_cs = [[None] * 4 for _ in range(4)]   # [k][cs]
    for k in (2, 3):
        for cs in range(4):
            mm_cs[k][cs] = pR1.tile([128, CS], F32R, tag=f"mm{k}c{cs}",
                                    name=f"mm{k}c{cs}")
    for k in (0, 1):
        for cs in range(4):
            mm_cs[k][cs] = pR2.tile([128, CS], F32R, tag=f"mm{k}c{cs}",
                                    name=f"mm{k}c{cs}")
    sp_sb = pR2.tile([8, N], F32R, tag="sp")
    for nm in ("img_wtT", "img_wpT", "img_wvT"):
        imgw[nm] = pR2.tile([128, KI, A], F32R, tag=nm, name=nm)

    def mm_cs_dma(cs):
        for k in range(4):
            nc.sync.dma_start(
                out=mm_cs[k][cs],
                in_=T["mm"][k * 128:(k + 1) * 128, cs * CS:(cs + 1) * CS].bitcast(F32R))

    nc.sync.dma_start(out=imgw["img_wtT"], in_=T["img_wtT"].bitcast(F32R))
    mm_cs_dma(0)
    nc.sync.dma_start(out=sp_sb, in_=T["mm"][512:520, :].bitcast(F32R))
    mm_cs_dma(1)
    nc.sync.dma_start(out=imgw["img_wpT"], in_=T["img_wpT"].bitcast(F32R))
    mm_cs_dma(2)
    nc.sync.dma_start(out=imgw["img_wvT"], in_=T["img_wvT"].bitcast(F32R))
    mm_cs_dma(3)

    def mm_ktile(k, cols):
        """[k-partitions, cols] slice of the mm operand for k-tile k.
        cols must lie within one CS-sized chunk for k < 4."""
        if k == 4:
            return sp_sb[:, cols]
        cs, lo, hi = cols.start // CS, cols.start % CS, None
        assert cols.stop - cols.start <= CS and cols.stop <= (cs + 1) * CS
        return mm_cs[k][cs][:, lo:lo + (cols.stop - cols.start)]

    def qkv_phase(branch, theta, phi, vt, wt, wp, wv, ks, bt2, bp2):
        """Computes theta [128,2,NC], phi [128,2,N], vt [128,MT,A] for one
        branch. ks = list of (ktile_idx, partitions).  theta/phi column
        blocks run as interleaved pairs so each weight load (serial ~139 ns
        inside the self-loading matmul) serves two 512-col streams."""
        nk = len(ks)
        for a2 in range(2):
            asl = slice(a2 * 128, (a2 + 1) * 128)
            for w, dst, b2, nblk in ((wt, theta, bt2, NSB), (wp, phi, bp2, N // NB)):
                for bp in range(nblk // 2):
                    csls = [slice((2 * bp + j) * NB, (2 * bp + j + 1) * NB)
                            for j in range(2)]
                    qs = [pps.tile([128, NB], F32, tag="blk", bufs=4,
                                   name=f"q{j}") for j in range(2)]
                    for i, (k, kp) in enumerate(ks):
                        for j in range(2):
                            nc.tensor.matmul(qs[j], lhsT=w[:kp, i, asl],
                                             rhs=mm_ktile(k, csls[j]),
                                             start=(i == 0), stop=(i == nk - 1))
                    for j in range(2):
                        nc.vector.tensor_scalar(out=dst[:, a2, csls[j]],
                                                in0=qs[j],
                                                scalar1=b2[:, a2:a2 + 1],
                                                scalar2=None, op0=ALU.add)
        for m in range(MT):
            msl = slice(m * 128, (m + 1) * 128)
            v_ps = pps.tile([128, A], F32, tag="blk", bufs=4, name="v_ps")
            for i, (k, kp) in enumerate(ks):
                nc.tensor.matmul(v_ps, lhsT=mm_ktile(k, msl)[:kp, :],
                                 rhs=wv[:kp, i, :],
                                 start=(i == 0), stop=(i == nk - 1))
            nc.vector.tensor_copy(out=vt[:, m, :], in_=v_ps)

    def attn_phase(branch, theta, phi, vt, pools):
        """Flash attention with the output conv folded into vt (PV emits
        G@att directly).  Two NB column blocks run interleaved through the
        m-loop so every phi/vt weight load (serial inside the self-loading
        matmul) serves two 512-col streams, and the PE always has
        dependency-free work while exp catches up.  Block 0's denominator
        accumulates on DVE, block 1's on Pool.  The per-pair tail
        (rs -> rcp -> bc -> normalize -> output) is emitted inside the NEXT
        pair's m-loop so the in-order PE never stalls on it."""
        fusT = pools["fus_sp2"]
        pR3 = pools["pR3"]
        pend = []

        def emit_out(pd):
            csl = slice(pd["nb"] * NB, (pd["nb"] + 1) * NB)
            for q2 in range(2):
                t1 = pR3.tile([128, NB], F16, tag="t1", bufs=4, name="t1")
                nc.vector.tensor_tensor(out=t1, in0=pd["att_ps"][:, q2, :],
                                        in1=pd["bc"], op=ALU.mult)
                if branch == "img":
                    nc.vector.tensor_tensor(out=part_out[:, q2, csl], in0=t1,
                                            in1=fusT[:, q2, csl], op=ALU.add)
                else:
                    out_t = pR3.tile([128, NB], F16, tag="out_t", bufs=2, name="out_t")
                    nc.vector.tensor_tensor(out=out_t, in0=t1,
                                            in1=part_out[:, q2, csl], op=ALU.add)
                    nc.sync.dma_start(
                        out=T["out"][q2 * 128:(q2 + 1) * 128, csl], in_=out_t)

        for nbp in range(NSB // 2):
            nbs = (2 * nbp, 2 * nbp + 1)
            csls = [slice(nb * NB, (nb + 1) * NB) for nb in nbs]
            att = [pps.tile([128, 2, NB], F32, tag="att", bufs=2,
                            name=f"att{j}") for j in range(2)]
            accs = [pR3.tile([128, NB], F32R, tag=f"acc{j}", bufs=2,
                             name=f"acc{j}") for j in range(2)]

            def pv(m_idx, pts):
                for a2 in range(2):
                    for j in range(2):
                        nc.tensor.matmul(
                            att[j][:, a2, :],
                            lhsT=vt[:, m_idx, a2 * 128:(a2 + 1) * 128],
                            rhs=pts[j],
                            start=(m_idx == 0), stop=(m_idx == MT - 1))

            prev_p = None
            for m in range(MT):
                msl = slice(m * 128, (m + 1) * 128)
                lt = [pps.tile([128, NB], F32, tag="blk", bufs=4,
                               name=f"lt{j}") for j in range(2)]
                for ka in range(2):
                    for j in range(2):
                        nc.tensor.matmul(lt[j], lhsT=phi[:, ka, msl],
                                         rhs=theta[:, ka, csls[j]],
                                         start=(ka == 0), stop=(ka == 1))
                if m < len(pend):
                    emit_out(pend[m])
                    if m == 1:
                        pend = []
                if prev_p is not None:
                    pv(m - 1, prev_p)
                ps = []
                for j in range(2):
                    p_sb = pR3.tile([128, NB], F32R, tag="p", bufs=6,
                                    name=f"p{j}")
                    nc.scalar.activation(out=p_sb, in_=lt[j], func=AF.Exp)
                    ps.append(p_sb)
                if m == 0:
                    nc.vector.tensor_copy(out=accs[0], in_=ps[0])
                    nc.gpsimd.tensor_copy(out=accs[1], in_=ps[1])
                else:
                    nc.vector.tensor_add(out=accs[0], in0=accs[0], in1=ps[0])
                    nc.gpsimd.tensor_add(out=accs[1], in0=accs[1], in1=ps[1])
                prev_p = ps
            pv(MT - 1, prev_p)
            # softmax denominator -> reciprocal -> broadcast (tail, pipelined)
            newpend = []
            for j in range(2):
                rs_ps = pps.tile([128, NB], F32, tag="blk", bufs=4,
                                 name=f"rs{j}")
                nc.tensor.matmul(rs_ps[0:1, :], lhsT=ones_r, rhs=accs[j],
                                 start=True, stop=True)
                rcp = pR3.tile([1, NB], F32, tag="rcp", bufs=4, name=f"rcp{j}")
                nc.vector.reciprocal(out=rcp, in_=rs_ps[0:1, :])
                bc = pR3.tile([128, NB], F32, tag="bc", bufs=4, name=f"bc{j}")
                nc.gpsimd.partition_broadcast(bc, rcp)
                newpend.append({"nb": nbs[j], "att_ps": att[j], "bc": bc})
            pend = newpend
        for pd in pend:
            emit_out(pd)

    # ---- img qkv -------------------------------------------------------
    theta = pL2.tile([128, 2, NC], F32R, tag="theta", name="theta_i")
    phi = pL2.tile([128, 2, N], F32R, tag="phi", name="phi_i")
    vt = pL2.tile([128, MT, A], F32R, tag="vt", name="vt_i")
    ks_img = [(0, 128), (1, 128), (2, 128), (3, 128), (4, 8)]
    qkv_phase("img", theta, phi, vt, imgw["img_wtT"], imgw["img_wpT"],
              imgw["img_wvT"], ks_img, bias_t["img_bt2"], bias_t["img_bp2"])
    pR2.release()

    # ---- working pool (attention + tails) ------------------------------
    pR3 = tc.alloc_tile_pool(name="work", bufs=1, side="right")
    pools = {"pR3": pR3}
    for nm, kt in (("lang_wtT", KL), ("lang_wpT", KL), ("lang_wvT", KL)):
        t = pR3.tile([128, kt, A], F32R, tag=nm, name=nm)
        nc.sync.dma_start(out=t, in_=T[nm].bitcast(F32R))
        pools[nm] = t
    fus_sp2 = pR3.tile([128, 2, NC], F16, tag="fus_sp2", name="fus_sp2")
    nc.sync.dma_start(out=fus_sp2, in_=T["fus_sp2"])
    pools["fus_sp2"] = fus_sp2

    # ---- img attention + partial fusion --------------------------------
    attn_phase("img", theta, phi, vt, pools)

    # ---- lang qkv ------------------------------------------------------
    theta_l = pL2.tile([128, 2, NC], F32R, tag="theta", name="theta_l")
    phi_l = pL2.tile([128, 2, N], F32R, tag="phi", name="phi_l")
    vt_l = pL2.tile([128, MT, A], F32R, tag="vt", name="vt_l")
    ks_lang = [(2, 128), (3, 128)]
    qkv_phase("lang", theta_l, phi_l, vt_l, pools["lang_wtT"],
              pools["lang_wpT"], pools["lang_wvT"], ks_lang,
              bias_t["lang_bt2"], bias_t["lang_bp2"])

    # ---- lang attention + final output ---------------------------------
    attn_phase("lang", theta_l, phi_l, vt_l, pools)

    pR3.release()
    pR1.release()
    pL2.release()
    pL1.release()
    pps.release()


def _build(repeat=1):
    nc = bacc.Bacc("TRN2", target_bir_lowering=False, debug=False, num_devices=8)
    T = {}
    T["mm"] = nc.dram_tensor("mm", [C_MM, N], F32, kind="ExternalInput").ap()
    for nm in ("img_wtT", "img_wpT", "img_wvT"):
        T[nm] = nc.dram_tensor(nm, [128, KI, A], F32, kind="ExternalInput").ap()
    for nm in ("lang_wtT", "lang_wpT", "lang_wvT"):
        T[nm] = nc.dram_tensor(nm, [128, KL, A], F32, kind="ExternalInput").ap()
    T["fus_sp2"] = nc.dram_tensor("fus_sp2", [128, 2, NC], F16,
                                  kind="ExternalInput").ap()
    for nm in ("img_bt2", "img_bp2", "lang_bt2", "lang_bp2"):
        T[nm] = nc.dram_tensor(nm, [128, 2], F32, kind="ExternalInput").ap()
    T["out"] = nc.dram_tensor("out", [A, NC], F16, kind="ExternalOutput").ap()

    with tile.TileContext(nc) as tc:
        for _ in range(repeat):
            _emit(nc, tc, T)
    nc.compile()
    return nc


def _spatial():
    gy, gx = np.meshgrid(np.linspace(0, 1, H, dtype=np.float32),
                         np.linspace(0, 1, W, dtype=np.float32), indexing="ij")
    feats = [gx, gy, 1.0 - gx, 1.0 - gy] + [(gx + gy) * 0.5] * 4
    return np.stack(feats[:8], axis=0).reshape(8, N).astype(np.float32)


def _pack_kT(wT, kt):
    """[C, A] (pre-transposed weight) -> [128, kt, A] partition-tiled."""
    out = np.zeros((128, kt, wT.shape[1]), np.float32)
    for k in range(kt):
        rows = wT[k * 128:min((k + 1) * 128, wT.shape[0])]
        out[:rows.shape[0], k] = rows
    return out


def _bias2(b):
    return np.ascontiguousarray(b.reshape(2, 128).T)


def _in_maps(inputs):
    f = lambda k: np.asarray(inputs[k], np.float32)
    images, flows = f("images"), f("flows")
    sp = _spatial()

    # fold wo through the fusion conv, then fold that G straight into wv so
    # the PV matmul emits G@att directly (no on-chip G stage); fold
    # bv/bo/spatial/fus_b into one additive field
    G_img = f("fus_w")[:, 0:256] @ f("img_wo")
    G_lang = f("fus_w")[:, 256:512] @ f("lang_wo")
    wv_eff_img = G_img @ f("img_wv")
    wv_eff_lang = G_lang @ f("lang_wv")
    bo_eff_img = f("img_wo") @ f("img_bv") + f("img_bo")
    bo_eff_lang = f("lang_wo") @ f("lang_bv") + f("lang_bo")
    fus_b_eff = (f("fus_b") + f("fus_w")[:, 0:256] @ bo_eff_img
                 + f("fus_w")[:, 256:512] @ bo_eff_lang)
    fus_sp_full = f("fus_w")[:, 512:520] @ sp + fus_b_eff[:, None]  # [256, N]

    base = {
        "img_wtT": _pack_kT(f("img_wt").T, KI),
        "img_wpT": _pack_kT(f("img_wp").T, KI),
        "img_wvT": _pack_kT(wv_eff_img.T, KI),
        "lang_wtT": _pack_kT(f("lang_wt").T, KL),
        "lang_wpT": _pack_kT(f("lang_wp").T, KL),
        "lang_wvT": _pack_kT(wv_eff_lang.T, KL),
        "img_bt2": _bias2(f("img_bt")),
        "img_bp2": _bias2(f("img_bp")),
        "lang_bt2": _bias2(f("lang_bt")),
        "lang_bp2": _bias2(f("lang_bp")),
    }

    in_maps = []
    for c in range(8):
        b, half = c // 2, c % 2
        mm = np.concatenate(
            [images[b].reshape(256, N), flows[b].reshape(256, N), sp], axis=0)
        if half:
            mm = np.roll(mm, -NC, axis=1)
        fsp = np.roll(fus_sp_full, -half * NC, axis=1)[:, :NC]
        fsp2 = np.ascontiguousarray(
            fsp.reshape(2, 128, NC).transpose(1, 0, 2).astype(np.float16))
        in_maps.append({**base, "mm": np.ascontiguousarray(mm), "fus_sp2": fsp2})
    return in_maps


def kernel(**inputs):
    if "nc" not in _CACHE:
        _CACHE["nc"] = _build()
    nc = _CACHE["nc"]
    in_maps = _in_maps(inputs)
    res = run_bass_kernel_spmd(nc, in_maps, list(range(8)))
    out = np.empty((B, A, N), np.float32)
    for c in range(8):
        b, half = c // 2, c % 2
        out[b][:, half * NC:(half + 1) * NC] = res.results[c]["out"].astype(np.float32)
    return out.reshape(B, A, H, W)

